# revision 3
# baseline (speedup 1.0000x reference)
"""Trainium2 Bass kernel for nn_ConvSparseKernel (sparse-tap conv, 5 taps).

Computation (per reference):
    Wn[k] = row-standardized W[k]  (per (k, out) row: subtract mean over in,
            then L2-normalize)
    y[b, :, oh, ow] = (sum_k Wn[k] @ x[b, :, oh+kh_k, ow+kw_k] + bias) * NF

Shapes (full): x [16, 256, 64, 64] f32, W [5, 256, 256] f32, bias [256] f32
Output: [16, 256, 62, 62] f32.

Sharding: data-parallel over batch -- 8 cores x 2 batches each; W/bias
replicated. Everything (standardization included) runs on-device.

Schedule (cost-model-driven; times are TimelineSim estimates):
  - PE must be continuously busy ~3us before it reaches 2.4 GHz, so a
    memset scratch tile feeds dummy transposes from ~0.35us (no identity
    dependency).  The real f32r matmul stream starts ~4.8us.
  - W taps load as 5 separate SWDGE DMAs on the gpsimd ring (25ns SEQ
    issue each); per-tap standardization (ACT square / DVE chain) runs as
    each tap lands, wn_k ready every ~1.0us.
  - x loads as row-pieces (20/14/14/16 rows) split ic0->SP ring,
    ic1->ACT ring, sized so each piece lands just before the matmul
    stream consumes it.  b1 x follows on the same rings.
  - prep0 runs tap-major blocks with growing chunk counts (t0/t1: c0-c1,
    t2-t4: c0-c2) matched to wn_k arrival; the PE transpose for tap k+1
    is tucked inside tap k's matmul block so its PSUM->SBUF copy never
    stalls the stream.
  - Main loop: per (b, oc, 8-row chunk) one PSUM bank accumulates 10
    f32r matmuls (N=496 -> 1 cycle/row); ACT applies bias*NF + scale and
    the y DMA drains on the gpsimd ring.  The last two drains use the
    ACT/SP HWDGE rings (625ns gen vs 1038ns SWDGE) to shorten the tail.
"""

import os

import numpy as np

KERNEL_KEYS = ((0, 0), (0, 2), (1, 1), (2, 0), (2, 2))
IN_CH = 256
OUT_CH = 256
H = 64
OH = 62
B_FULL = 16
N_CORES = 8
B_LOCAL = B_FULL // N_CORES
NF = float(1.0 / np.sqrt(IN_CH * len(KERNEL_KEYS) + 1))
ROW_CHUNK = 8  # rows of output per PSUM tile -> N = 8*62 = 496 <= 512

# x row-piece boundaries for batch 0 (streamed ahead of the matmuls) and
# batch 1 (bulk, latency-insensitive).
ROWS_B0 = ((0, 20), (20, 34), (34, 48), (48, 64))
ROWS_B1 = ((0, 32), (32, 64))

# Dummy-transpose warmup calibration (see _emit).
DUM_BIG = int(os.environ.get("DUM_BIG", "30"))
DUM_SMALL = int(os.environ.get("DUM_SMALL", "6"))
DUM_SPLICE = int(os.environ.get("DUM_SPLICE", "6"))

_compiled_nc = None


def _emit(tc, nc, y, x, w, bias):
    import concourse.mybir as mybir
    from concourse.masks import make_identity

    f32 = mybir.dt.float32
    f32r = mybir.dt.float32r
    AF = mybir.ActivationFunctionType
    AX = mybir.AxisListType
    NTAP = len(KERNEL_KEYS)

    w_okI = w.rearrange("k o i -> o k i")
    bias2d = bias.rearrange("(p u) -> p u", u=1)

    with tc.tile_pool(name="const", bufs=1) as cpool, \
         tc.tile_pool(name="wprep", bufs=1) as wpool, \
         tc.tile_pool(name="tpsum", bufs=2, space="PSUM") as tpool, \
         tc.tile_pool(name="mmpsum", bufs=5, space="PSUM") as mpool, \
         tc.tile_pool(name="outp", bufs=12) as opool:

        # ---- SBUF tiles ----
        junk = cpool.tile([64, 64], f32, name="junk")
        ident_f32 = cpool.tile([128, 128], f32, name="ident_f32")
        ident = cpool.tile([128, 128], f32r, name="ident")
        sqrt_warm = cpool.tile([64, 1], f32, name="sqrt_warm")
        wraw = [cpool.tile([128, NTAP, IN_CH], f32, name=f"wraw_{oc}",
                           tag=f"wraw_{oc}") for oc in range(2)]
        braw = [cpool.tile([128, 1], f32, name=f"braw_{oc}",
                           tag=f"braw_{oc}") for oc in range(2)]
        bnf = [cpool.tile([128, 1], f32, name=f"bnf_{oc}", tag=f"bnf_{oc}")
               for oc in range(2)]
        wn = [wpool.tile([128, NTAP, IN_CH], f32r, name=f"wn_{oc}",
                         tag=f"wn_{oc}") for oc in range(2)]
        wt = [cpool.tile([128, NTAP, 2, 128], f32r, name=f"wt_{oc}",
                         tag=f"wt_{oc}") for oc in range(2)]
        xt = [[cpool.tile([128, H, H], f32r, name=f"xt_{b}_{cc}",
                          tag=f"xt_{b}_{cc}") for cc in range(2)]
              for b in range(B_LOCAL)]
        st = {}
        for oc in range(2):
            for nm in ("ssq", "sums", "mu", "musums", "var", "sd", "inv"):
                st[(oc, nm)] = wpool.tile([128, NTAP], f32,
                                          name=f"{nm}_{oc}",
                                          tag=f"{nm}_{oc}")
            st[(oc, "sqs")] = wpool.tile([128, IN_CH], f32,
                                         name=f"sqs_{oc}", tag=f"sqs_{oc}")

        # ---- Pool (gpsimd/SWDGE) ring: scratch memset, W taps, identity,
        # remaining W/bias, later the y drains.  SEQ issue is only 25ns
        # here, and the SWDGE gens run on the otherwise-idle Pool engine.
        nc.gpsimd.memset(junk, 1.0)
        nc.gpsimd.dma_start(out=wraw[0][:, 0, :], in_=w_okI[0:128, 0, :])
        make_identity(nc, ident_f32)
        for k in range(1, NTAP):
            nc.gpsimd.dma_start(out=wraw[0][:, k, :], in_=w_okI[0:128, k, :])
        nc.gpsimd.dma_start(out=braw[0], in_=bias2d[0:128])
        nc.gpsimd.dma_start(out=wraw[1], in_=w_okI[128:256])
        nc.gpsimd.dma_start(out=braw[1], in_=bias2d[128:256])

        # ---- ACT (HWDGE) ring: sqrt-table warm first, then b0.ic1 row
        # pieces; b1.ic1 comes later (emitted after the oc0 stats).
        nc.scalar.sqrt(sqrt_warm, junk[:, 0:1])
        for (r0, r1) in ROWS_B0:
            nc.scalar.dma_start(out=xt[0][1][:, r0:r1, :],
                                in_=x[0, 128:256, r0:r1, :])

        # ---- SP (HWDGE) ring: b0.ic0 pieces then b1.ic0 halves; the very
        # last y drain is appended at the end of emission.
        for (r0, r1) in ROWS_B0:
            nc.sync.dma_start(out=xt[0][0][:, r0:r1, :],
                              in_=x[0, 0:128, r0:r1, :])
        for (r0, r1) in ROWS_B1:
            nc.sync.dma_start(out=xt[1][0][:, r0:r1, :],
                              in_=x[1, 0:128, r0:r1, :])

        # ---- DVE: identity f32->f32r round-copy first (ident_f32 lands
        # ~1.9us; real transposes need it ~4.1us), then the stats chains.
        nc.vector.tensor_copy(out=ident, in_=ident_f32)

        # ---- PE warmup: dummy transposes on the memset scratch keep PE
        # continuously busy from ~0.35us so the p-state ramp (3us) is done
        # before the real stream starts.  [64,64] f32 transposes are 128
        # PE cycles each; the [32,32] tail gives fine splice granularity.
        def dummy(n, small=False):
            for _ in range(n):
                dt_ = tpool.tile([64, 64], f32, name="dum", tag="dum",
                                 bufs=1)
                if small:
                    nc.tensor.transpose(dt_[0:32, 0:32], junk[0:32, 0:32],
                                        junk[0:32, 0:32])
                else:
                    nc.tensor.transpose(dt_, junk, junk)

        dummy(DUM_BIG)
        dummy(DUM_SMALL, small=True)

        # ---- weight standardization (per tap) ----
        # ||w - mu||^2 = ssq - mu*sums, so sq/ssq don't wait on the mean.
        def stats_tap(oc, k):
            ks = slice(k, k + 1)
            # ssq_k = sum(w_k^2) on ACT (Square + accum), off the DVE
            # chain. (tensor_tensor_reduce wedges TRN2 here.)
            nc.scalar.activation(st[(oc, "sqs")], wraw[oc][:, k, :],
                                 AF.Square, accum_out=st[(oc, "ssq")][:, ks])
            nc.vector.reduce_sum(out=st[(oc, "sums")][:, ks],
                                 in_=wraw[oc][:, k, :], axis=AX.X)
            nc.vector.tensor_scalar_mul(st[(oc, "mu")][:, ks],
                                        st[(oc, "sums")][:, ks], 1.0 / IN_CH)
            nc.vector.tensor_mul(out=st[(oc, "musums")][:, ks],
                                 in0=st[(oc, "mu")][:, ks],
                                 in1=st[(oc, "sums")][:, ks])
            nc.vector.tensor_sub(out=st[(oc, "var")][:, ks],
                                 in0=st[(oc, "ssq")][:, ks],
                                 in1=st[(oc, "musums")][:, ks])
            nc.scalar.sqrt(st[(oc, "sd")][:, ks], st[(oc, "var")][:, ks])
            nc.vector.reciprocal(st[(oc, "inv")][:, ks],
                                 st[(oc, "sd")][:, ks])
            # wn_k = (w_k - mu_k) * inv_k, one fused DVE op
            nc.vector.tensor_scalar(
                out=wn[oc][:, k, :], in0=wraw[oc][:, k, :],
                scalar1=st[(oc, "mu")][:, ks],
                scalar2=st[(oc, "inv")][:, ks],
                op0=mybir.AluOpType.subtract,
                op1=mybir.AluOpType.mult)

        def transpose_tap(oc, k):
            for ic in range(2):
                pt = tpool.tile([128, 128], f32r, name="pt", tag="pt")
                nc.tensor.transpose(
                    pt, wn[oc][:, k, ic * 128:(ic + 1) * 128], ident)
                # alternate PSUM->SBUF copy engine: DVE / ACT
                if ic == 0:
                    nc.vector.tensor_copy(out=wt[oc][:, k, ic, :], in_=pt)
                else:
                    nc.scalar.copy(wt[oc][:, k, ic, :], pt)

        # ---- main-loop helpers ----
        # Per-chunk PSUM tiles accumulate 10 matmuls; prep0 spreads them
        # across tap-major blocks so start/stop flags are tracked per
        # chunk.
        chunk_ps = {}
        chunk_cnt = {}

        def mm(b, oc, c, k, ic):
            key = (b, oc, c)
            r0 = c * ROW_CHUNK
            nr = min(ROW_CHUNK, OH - r0)
            if key not in chunk_ps:
                chunk_ps[key] = mpool.tile([128, nr, OH], f32, name="ps",
                                           tag="ps")
                chunk_cnt[key] = 0
            idx = chunk_cnt[key]
            kh, kw = KERNEL_KEYS[k]
            rhs = xt[b][ic][:, kh + r0:kh + r0 + nr, kw:kw + OH]
            nc.tensor.matmul(chunk_ps[key], wt[oc][:, k, ic, :], rhs,
                             start=(idx == 0), stop=(idx == 2 * NTAP - 1))
            chunk_cnt[key] = idx + 1

        drain_ring = [None]

        def drain_chunk(b, oc, c):
            key = (b, oc, c)
            assert chunk_cnt[key] == 2 * NTAP
            r0 = c * ROW_CHUNK
            nr = min(ROW_CHUNK, OH - r0)
            ot = opool.tile([128, nr, OH], f32, name="ot", tag="ot")
            nc.scalar.activation(ot, chunk_ps[key], AF.Identity,
                                 bias=bnf[oc], scale=NF)
            eng = drain_ring[0] or nc.gpsimd
            eng.dma_start(
                out=y[b, oc * 128:(oc + 1) * 128, r0:r0 + nr, :], in_=ot)
            del chunk_ps[key], chunk_cnt[key]

        def conv_chunk(b, oc, c):
            for k in range(NTAP):
                for ic in range(2):
                    mm(b, oc, c, k, ic)
            drain_chunk(b, oc, c)

        NCH = (OH + ROW_CHUNK - 1) // ROW_CHUNK  # 8 chunks (last is 6 rows)

        # ---- prep0: oc0 stats/transposes fused with the first chunks'
        # matmuls, block sizes matched to the per-tap W arrival cadence.
        # Emission order below is per-engine program order.
        stats_tap(0, 0)
        transpose_tap(0, 0)
        dummy(DUM_SPLICE)            # covers the wt0 PSUM->SBUF copy
        stats_tap(0, 1)
        mm(0, 0, 0, 0, 0)
        mm(0, 0, 0, 0, 1)
        stats_tap(0, 2)
        transpose_tap(0, 1)          # trans t1 inside t0 block
        mm(0, 0, 1, 0, 0)
        mm(0, 0, 1, 0, 1)
        # t1 block (c0, c1) with trans t2 in the middle
        mm(0, 0, 0, 1, 0)
        mm(0, 0, 0, 1, 1)
        transpose_tap(0, 2)
        stats_tap(0, 3)
        mm(0, 0, 1, 1, 0)
        mm(0, 0, 1, 1, 1)
        # t2 block (c0..c2) with trans t3 in the middle
        mm(0, 0, 0, 2, 0)
        mm(0, 0, 0, 2, 1)
        mm(0, 0, 1, 2, 0)
        transpose_tap(0, 3)
        stats_tap(0, 4)
        mm(0, 0, 1, 2, 1)
        mm(0, 0, 2, 2, 0)
        mm(0, 0, 2, 2, 1)
        # t3 block (c0..c2) with trans t4 in the middle
        mm(0, 0, 0, 3, 0)
        mm(0, 0, 0, 3, 1)
        mm(0, 0, 1, 3, 0)
        transpose_tap(0, 4)
        mm(0, 0, 1, 3, 1)
        mm(0, 0, 2, 3, 0)
        mm(0, 0, 2, 3, 1)
        # t4 block (c0..c2)
        mm(0, 0, 0, 4, 0)
        mm(0, 0, 0, 4, 1)
        mm(0, 0, 1, 4, 0)
        mm(0, 0, 1, 4, 1)
        mm(0, 0, 2, 4, 0)
        mm(0, 0, 2, 4, 1)
        # c2 completion (taps 0, 1)
        mm(0, 0, 2, 0, 0)
        mm(0, 0, 2, 0, 1)
        mm(0, 0, 2, 1, 0)
        mm(0, 0, 2, 1, 1)
        # bnf0 on ACT before the first drain activation
        nc.scalar.mul(bnf[0], braw[0], NF)
        drain_chunk(0, 0, 0)
        drain_chunk(0, 0, 1)
        drain_chunk(0, 0, 2)

        # b1.ic1 bulk loads now that the early ACT-ring pieces are out.
        for (r0, r1) in ROWS_B1:
            nc.scalar.dma_start(out=xt[1][1][:, r0:r1, :],
                                in_=x[1, 128:256, r0:r1, :])
        nc.scalar.mul(bnf[1], braw[1], NF)

        # oc1 stats (W oc1 lands ~11us; transposes are interleaved into
        # the c3..c7 chunk stream below).
        for k in range(NTAP):
            stats_tap(1, k)

        # rest of b0.oc0, with oc1 transposes tucked between chunks
        for c in range(3, NCH):
            conv_chunk(0, 0, c)
            if c - 3 < NTAP:
                transpose_tap(1, c - 3)

        for c in range(NCH):
            conv_chunk(0, 1, c)
        for c in range(NCH):
            conv_chunk(1, 0, c)
        for c in range(NCH):
            if c == NCH - 2:
                drain_ring[0] = nc.scalar
            elif c == NCH - 1:
                drain_ring[0] = nc.sync
            conv_chunk(1, 1, c)


def _build_nc():
    import concourse.mybir as mybir
    import concourse.tile as tile
    from concourse import bacc

    f32 = mybir.dt.float32
    f32r = mybir.dt.float32r
    nc = bacc.Bacc("TRN2", target_bir_lowering=False, debug=False)
    x = nc.dram_tensor("x", (B_LOCAL, IN_CH, H, H), f32r,
                       kind="ExternalInput").ap()
    w = nc.dram_tensor("w", (len(KERNEL_KEYS), OUT_CH, IN_CH), f32,
                       kind="ExternalInput").ap()
    bias = nc.dram_tensor("bias", (OUT_CH,), f32, kind="ExternalInput").ap()
    y = nc.dram_tensor("y", (B_LOCAL, OUT_CH, OH, OH), f32,
                       kind="ExternalOutput").ap()

    with tile.TileContext(nc) as tc:
        _emit(tc, nc, y, x, w, bias)
    nc.compile()
    return nc


def _get_nc():
    global _compiled_nc
    if _compiled_nc is None:
        _compiled_nc = _build_nc()
    return _compiled_nc


def _make_in_maps(x, W, bias):
    x = np.ascontiguousarray(x, dtype=np.float32)
    W = np.ascontiguousarray(W, dtype=np.float32)
    bias = np.ascontiguousarray(bias, dtype=np.float32)
    return [
        {
            "x": np.ascontiguousarray(x[i * B_LOCAL:(i + 1) * B_LOCAL]),
            "w": W,
            "bias": bias,
        }
        for i in range(N_CORES)
    ]


def kernel(x, W, bias):
    from concourse import bass_utils

    nc = _get_nc()
    res = bass_utils.run_bass_kernel_spmd(
        nc, _make_in_maps(x, W, bias), core_ids=list(range(N_CORES)))
    return np.concatenate([r["y"] for r in res.results], axis=0)
